# revision 2
# baseline (speedup 1.0000x reference)
"""Trainium2 Bass kernel for a 2-layer GRU time-series binary classifier, v2.

Model (torch GRU semantics, batch_first):
  seq1, _ = GRU(F=2048 -> H1=128)(x)        x: [64, 512, 2048]
  _,  h2 = GRU(H1 -> H2=64)(seq1)
  out = h2 @ fc_w.T + fc_b                  -> [64, 1]

Data-parallel over batch across 8 cores (8 sequences each).  The wall
time is the 512-step recurrence chain latency, so the per-step
cross-engine dependency chain is minimized:

  PE(3 bf16 matmuls) -> ACT sigmoid(r|zeta) -> DVE stt -> DVE tt
     -> ACT tanh -> DVE tensor_tensor_scan -> next PE

Key structure:
  * z' = 1-z comes free as sigmoid(-zeta) via the ACT scale port.
  * h' = n*(1-z) + z*h is ONE DVE op: a hardware prefix scan over
    per-sequence column pairs d0=[0, n], d1=[z', z*h]; the zero in d0
    kills the carried state at each pair boundary, so
    out[odd] = n*z' + z*h exactly.  z*h is computed off the critical
    path on DVE as soon as sigmoid lands (never GpSimd: its semaphore
    instructions cost ~1.3us each on HW).
  * All matmul operands are bf16 (h history stored bf16) so LDWEIGHTS
    costs halve; accumulation stays fp32 in PSUM, gates/cell fp32.
  * PSUM discipline: a start=True matmul clears has_written bits for
    its WHOLE bank, and the tile framework only tracks region-level
    deps, so every start=True producer (per-chunk GEMM rz, per-step
    hn, per-layer xn scratch) owns a full, exclusive bank.
Layer 2 runs LAG slots behind layer 1, brick-laid one engine-phase
behind inside each slot; the chunked input GEMMs ride as low-priority
thunks in the PE's idle windows.
"""

import numpy as np
import ml_dtypes

from concourse import bacc, tile, mybir
from concourse.bass_utils import run_bass_kernel_spmd

BF16 = ml_dtypes.bfloat16
N_CORES = 8
B, T, F = 64, 512, 2048
import os as _os
T = int(_os.environ.get("BASS_GRU_T", T))   # shrink for simulator checks
H1, H2 = 128, 64
BL = B // N_CORES             # 8 sequences per core
CHUNK = 32                    # timesteps per GEMM chunk
NCH = T // CHUNK              # 16 chunks
NW = CHUNK * BL               # 256 moving columns per chunk GEMM
KT = F // 128                 # 16 K-tiles for GEMM1
LAG = 40                      # layer-2 slot lag
R = 8                         # step ring size
AF = mybir.ActivationFunctionType
ALU = mybir.AluOpType
DT_BF = mybir.dt.bfloat16
DT_F32 = mybir.dt.float32


def build_nc():
    nc = bacc.Bacc(None, target_bir_lowering=False)

    xT = nc.declare_dram_parameter("xT", [F, T, BL], DT_BF, isOutput=False)
    wih1T = nc.declare_dram_parameter("wih1T", [F, 3 * H1], DT_BF, isOutput=False)
    whh1T = nc.declare_dram_parameter("whh1T", [H1, 3 * H1], DT_BF, isOutput=False)
    wih2T = nc.declare_dram_parameter("wih2T", [H1, 3 * H2], DT_BF, isOutput=False)
    whh2T = nc.declare_dram_parameter("whh2T", [H2, 3 * H2], DT_BF, isOutput=False)
    brow1 = nc.declare_dram_parameter("brow1", [1, 3 * H1], DT_F32, isOutput=False)
    brow2 = nc.declare_dram_parameter("brow2", [1, 3 * H2], DT_F32, isOutput=False)
    bhn1 = nc.declare_dram_parameter("bhn1", [H1, 1], DT_F32, isOutput=False)
    bhn2 = nc.declare_dram_parameter("bhn2", [H2, 1], DT_F32, isOutput=False)
    fcwT = nc.declare_dram_parameter("fcwT", [H2, 1], DT_F32, isOutput=False)
    fcb = nc.declare_dram_parameter("fcb", [BL, 1], DT_F32, isOutput=False)
    out = nc.declare_dram_parameter("out", [BL, 1], DT_F32, isOutput=True)

    with tile.TileContext(nc) as tc:
        with (
            tc.tile_pool(name="const", bufs=1) as cpool,
            tc.tile_pool(name="xchunk", bufs=3) as xpool,
            tc.tile_pool(name="g1p", bufs=2, space="PSUM") as g1pool,
            tc.tile_pool(name="g2p", bufs=2, space="PSUM") as g2pool,
            tc.tile_pool(name="scr", bufs=1, space="PSUM") as scrpool,
            tc.tile_pool(name="pfix", bufs=1, space="PSUM") as pfix,
        ):
            # ---- persistent tiles -------------------------------------
            w1 = cpool.tile([128, KT, 3 * H1], DT_BF)      # GEMM1 stationaries
            wh1 = cpool.tile([H1, 3 * H1], DT_BF)
            w2 = cpool.tile([H1, 3 * H2], DT_BF)
            wh2 = cpool.tile([H2, 3 * H2], DT_BF)
            br1 = cpool.tile([1, 3 * H1], DT_F32)
            br2 = cpool.tile([1, 3 * H2], DT_F32)
            bn1 = cpool.tile([H1, 1], DT_F32)
            bn2 = cpool.tile([H2, 1], DT_F32)
            fw = cpool.tile([H2, 1], DT_F32)
            fb = cpool.tile([BL, 1], DT_F32)
            onesf = cpool.tile([1, NW], DT_F32)
            xn1s = cpool.tile([H1, 2, NW], DT_F32)         # drained xn, A/B
            xn2s = cpool.tile([H2, 2, NW], DT_F32)

            # h1 history: per step a [2, BL] pair block; odd cols hold h.
            h1h = cpool.tile([H1, (T + 1) * 2 * BL], DT_BF)
            h2r = cpool.tile([H2, R * 2 * BL], DT_BF)      # h2 ring
            # step rings (slot = step % R)
            trz1 = cpool.tile([H1, R, 2, BL], DT_F32)      # [r, z]
            trz2 = cpool.tile([H2, R, 2, BL], DT_F32)
            d0r1 = cpool.tile([H1, R * 2 * BL], DT_F32)    # [0, n]
            d0r2 = cpool.tile([H2, R * 2 * BL], DT_F32)
            d1r1 = cpool.tile([H1, R * 2 * BL], DT_F32)    # [z', z*h]
            d1r2 = cpool.tile([H2, R * 2 * BL], DT_F32)
            m1 = cpool.tile([H1, R, BL], DT_F32)           # (hn+bn)*r
            m2 = cpool.tile([H2, R, BL], DT_F32)
            h2fin = cpool.tile([H2, BL], DT_F32)
            res = cpool.tile([BL, 1], DT_F32)

            # recurrent-n rings (slot = step % RN), one bank per layer:
            # a start=True matmul clears has_written for its WHOLE bank, so
            # the two layers' per-step fresh matmuls must not share one.
            RN = 32
            pband1 = pfix.tile([H1, 512], DT_F32)
            pband2 = pfix.tile([H2, 512], DT_F32)
            hn1 = pband1[:, 0:RN * BL]
            hn2 = pband2[:, 0:RN * BL]
            fcp = pband2[0:BL, RN * BL + 128:RN * BL + 129]
            # per-layer scratch banks for the xn GEMM outputs (a start=True
            # matmul wipes its whole bank, so the layers must not share one;
            # full-bank sized so the allocator cannot co-locate them)
            scr1 = scrpool.tile([H1, 512], DT_F32)
            scr2 = scrpool.tile([H2, 512], DT_F32)

            nc.sync.dma_start(out=w1[:], in_=wih1T.rearrange("(kt p) g -> p kt g", p=128))
            nc.sync.dma_start(out=wh1[:], in_=whh1T[:])
            nc.sync.dma_start(out=w2[:], in_=wih2T[:])
            nc.sync.dma_start(out=wh2[:], in_=whh2T[:])
            nc.sync.dma_start(out=br1[:], in_=brow1[:])
            nc.sync.dma_start(out=br2[:], in_=brow2[:])
            nc.sync.dma_start(out=bn1[:], in_=bhn1[:])
            nc.sync.dma_start(out=bn2[:], in_=bhn2[:])
            nc.sync.dma_start(out=fw[:], in_=fcwT[:])
            nc.sync.dma_start(out=fb[:], in_=fcb[:])
            nc.vector.memset(onesf[:], 1.0)
            nc.vector.memset(d0r1[:], 0.0)   # even cols must stay 0 forever
            nc.vector.memset(d0r2[:], 0.0)
            nc.vector.memset(h1h[:, 0:2 * BL], 0.0)   # h1(-1) = 0
            nc.vector.memset(h2r[:, 0:2 * BL], 0.0)   # h2(-1) = 0

            # paired views: [..., region/slot, b, {even,odd}]
            h1v = h1h.rearrange("p (g b two) -> p g b two", b=BL, two=2)
            h2v = h2r.rearrange("p (g b two) -> p g b two", b=BL, two=2)
            d0v1 = d0r1.rearrange("p (g b two) -> p g b two", b=BL, two=2)
            d0v2 = d0r2.rearrange("p (g b two) -> p g b two", b=BL, two=2)
            d1v1 = d1r1.rearrange("p (g b two) -> p g b two", b=BL, two=2)
            d1v2 = d1r2.rearrange("p (g b two) -> p g b two", b=BL, two=2)

            xtiles = {}
            rz1_ps = {}
            rz2_ps = {}
            xn1_ps = {}
            xn2_ps = {}

            def dma_xchunk(c):
                # split across two HWDGE rings (SP + ACT) so the ~47us/MB
                # single-DMA-engine transfer doesn't starve the GEMM riders
                xt = xpool.tile([128, KT, NW], DT_BF, tag="xc")
                src = xT[:, c * CHUNK:(c + 1) * CHUNK, :].rearrange(
                    "(kt p) t b -> p kt (t b)", p=128)
                nc.sync.dma_start(out=xt[:, 0:KT // 2], in_=src[:, 0:KT // 2])
                nc.scalar.dma_start(out=xt[:, KT // 2:KT],
                                    in_=src[:, KT // 2:KT])
                xtiles[c] = xt

            def gemm1_closures(c):
                """Layer-1 input projection of chunk c -> list of thunks."""
                rz = g1pool.tile([H1, 2 * NW], DT_F32, tag="g1")
                xn = scr1[:, 0:NW]
                rz1_ps[c], xn1_ps[c] = rz, xn1s[:, c % 2]
                xt = xtiles[c]
                thunks = []
                for g, dst, st0 in ((0, rz[:, 0:NW], True), (1, rz[:, NW:2 * NW], False),
                                    (2, xn[:, 0:NW], True)):
                    def mk(kt, g=g, dst=dst, st0=st0):
                        def f():
                            # only a bank's FIRST matmul may set start=True
                            nc.tensor.matmul(
                                dst, w1[:, kt, g * 128:(g + 1) * 128], xt[:, kt],
                                start=(st0 and kt == 0), stop=False,
                                skip_group_check=True)
                        return f
                    for kt in range(KT):
                        thunks.append(mk(kt))

                    def fbias(g=g, dst=dst):
                        nc.tensor.matmul(
                            dst, br1[:, g * 128:(g + 1) * 128], onesf[:],
                            start=False, stop=True, skip_group_check=True)
                    thunks.append(fbias)

                def fdrain():
                    nc.scalar.copy(xn1s[:, c % 2], xn[:, 0:NW])
                thunks.append(fdrain)
                return thunks

            def gemm2_closures(c):
                """Layer-2 input projection of chunk c (reads h1 history)."""
                rz = g2pool.tile([H2, 2 * NW], DT_F32, tag="g2")
                xn = scr2[:, 0:NW]
                rz2_ps[c], xn2_ps[c] = rz, xn2s[:, c % 2]
                mv = h1v[:, c * CHUNK + 1:(c + 1) * CHUNK + 1, :, 1]  # [H1,32,8]
                thunks = []
                for g, dst, st0 in ((0, rz[:, 0:NW], True), (1, rz[:, NW:2 * NW], False),
                                    (2, xn[:, 0:NW], True)):
                    def fmm(g=g, dst=dst, st0=st0):
                        nc.tensor.matmul(
                            dst, w2[:, g * H2:(g + 1) * H2], mv,
                            start=st0, stop=False, skip_group_check=True)
                    thunks.append(fmm)

                    def fbias(g=g, dst=dst):
                        nc.tensor.matmul(
                            dst, br2[:, g * H2:(g + 1) * H2], onesf[:],
                            start=False, stop=True, skip_group_check=True)
                    thunks.append(fbias)

                def fdrain():
                    nc.scalar.copy(xn2s[:, c % 2], xn[:, 0:NW])
                thunks.append(fdrain)
                return thunks

            def step_phases(layer, s):
                c, t, k32, k = s // CHUNK, s % CHUNK, s % RN, s % R
                if layer == 1:
                    H, wh, bn, rz, xn, hn = H1, wh1, bn1, rz1_ps[c], xn1_ps[c], hn1
                    trz, d0, d0v, d1, d1v, mt = trz1, d0r1, d0v1, d1r1, d1v1, m1
                    hprev = h1v[:, s, :, 1]                # [H, BL] h(s-1)
                    hout = h1h[:, (s + 1) * 2 * BL:(s + 2) * 2 * BL]
                else:
                    H, wh, bn, rz, xn, hn = H2, wh2, bn2, rz2_ps[c], xn2_ps[c], hn2
                    trz, d0, d0v, d1, d1v, mt = trz2, d0r2, d0v2, d1r2, d1v2, m2
                    hprev = h2v[:, s % R, :, 1]
                    hout = h2r[:, ((s + 1) % R) * 2 * BL:(((s + 1) % R) + 1) * 2 * BL]
                hnsl = hn[:, k32 * BL:(k32 + 1) * BL]
                rsl = slice(t * BL, (t + 1) * BL)
                zsl = slice(NW + t * BL, NW + (t + 1) * BL)

                def ph_mm():
                    nc.tensor.matmul(rz[:, rsl], wh[:, 0:H], hprev,
                                     start=False, stop=True, skip_group_check=True)
                    nc.tensor.matmul(rz[:, zsl], wh[:, H:2 * H], hprev,
                                     start=False, stop=True, skip_group_check=True)
                    nc.tensor.matmul(hnsl, wh[:, 2 * H:3 * H], hprev,
                                     start=True, stop=True, skip_group_check=True)

                def ph_sig():
                    rzv = rz.rearrange("p (g x) -> p g x", g=2)[:, :, rsl]
                    nc.scalar.activation(trz[:, k], rzv, AF.Sigmoid)
                    # z' = 1 - z = sigmoid(-zeta) into d1 even cols
                    nc.scalar.activation(d1v[:, k, :, 0], rz[:, zsl],
                                         AF.Sigmoid, scale=-1.0)

                def ph_b():
                    # z*h into d1 odd cols (off critical path)
                    nc.vector.tensor_tensor(out=d1v[:, k, :, 1], in0=trz[:, k, 1],
                                            in1=hprev, op=ALU.mult)

                def ph_mul():
                    nc.vector.scalar_tensor_tensor(
                        out=mt[:, k], in0=hnsl, scalar=bn[:], in1=trz[:, k, 0],
                        op0=ALU.add, op1=ALU.mult)
                    nc.vector.tensor_tensor(out=hnsl, in0=mt[:, k],
                                            in1=xn[:, rsl], op=ALU.add)

                def ph_tanh():
                    nc.scalar.activation(d0v[:, k, :, 1], hnsl, AF.Tanh)

                def ph_scan():
                    nc.vector.tensor_tensor_scan(
                        out=hout, data0=d0[:, k * 2 * BL:(k + 1) * 2 * BL],
                        data1=d1[:, k * 2 * BL:(k + 1) * 2 * BL],
                        initial=0.0, op0=ALU.mult, op1=ALU.add)

                return {"mm": ph_mm, "sig": ph_sig, "b": ph_b, "mul": ph_mul,
                        "tanh": ph_tanh, "scan": ph_scan}

            # ---- prologue --------------------------------------------
            dma_xchunk(0)
            dma_xchunk(1)
            for f in gemm1_closures(0):
                f()

            # ---- flat slot timeline ----------------------------------
            thunks = []
            for s in range(T + LAG):
                if s % CHUNK == 0:
                    c = s // CHUNK
                    if 1 <= c <= NCH:
                        thunks += gemm2_closures(c - 1)
                    if c + 2 < NCH:
                        thunks.append(lambda c=c: dma_xchunk(c + 2))
                    if c + 1 < NCH:
                        thunks += gemm1_closures(c + 1)
                u = s - LAG
                p1 = step_phases(1, s) if s < T else None
                p2 = step_phases(2, u) if 0 <= u < T else None
                for ph in ("mm", "sig", "mul", "b", "tanh", "scan"):
                    if p1:
                        p1[ph]()
                    if p2:
                        p2[ph]()
                for _ in range(2):
                    if thunks:
                        thunks.pop(0)()
            while thunks:
                thunks.pop(0)()

            # ---- fc head ---------------------------------------------
            nc.scalar.copy(h2fin[:], h2v[:, T % R, :, 1])
            nc.tensor.matmul(fcp, h2fin[:], fw[:], start=True, stop=True,
                             skip_group_check=True)
            nc.scalar.activation(res[:], fcp, AF.Identity, bias=fb[:])
            nc.sync.dma_start(out=out[:], in_=res[:])

    nc.compile()
    return nc


_NC_CACHE = {}


def _get_nc():
    if "nc" not in _NC_CACHE:
        _NC_CACHE["nc"] = build_nc()
    return _NC_CACHE["nc"]


def _prep_maps(x, w_ih1, w_hh1, b_ih1, b_hh1, w_ih2, w_hh2, b_ih2, b_hh2,
               fc_w, fc_b):
    f32 = np.float32
    brow1v = np.concatenate([
        (b_ih1[:H1] + b_hh1[:H1]),
        (b_ih1[H1:2 * H1] + b_hh1[H1:2 * H1]),
        b_ih1[2 * H1:],
    ]).reshape(1, 3 * H1)
    brow2v = np.concatenate([
        (b_ih2[:H2] + b_hh2[:H2]),
        (b_ih2[H2:2 * H2] + b_hh2[H2:2 * H2]),
        b_ih2[2 * H2:],
    ]).reshape(1, 3 * H2)
    shared = {
        "wih1T": np.ascontiguousarray(w_ih1.T).astype(BF16),
        "whh1T": np.ascontiguousarray(w_hh1.T).astype(BF16),
        "wih2T": np.ascontiguousarray(w_ih2.T).astype(BF16),
        "whh2T": np.ascontiguousarray(w_hh2.T).astype(BF16),
        "brow1": brow1v.astype(f32),
        "brow2": brow2v.astype(f32),
        "bhn1": np.ascontiguousarray(b_hh1[2 * H1:].reshape(H1, 1), dtype=f32),
        "bhn2": np.ascontiguousarray(b_hh2[2 * H2:].reshape(H2, 1), dtype=f32),
        "fcwT": np.ascontiguousarray(fc_w.reshape(1, H2).T, dtype=f32),
        "fcb": np.full((BL, 1), float(np.asarray(fc_b).reshape(-1)[0]), dtype=f32),
    }
    maps = []
    for c in range(N_CORES):
        xc = x[c * BL:(c + 1) * BL]          # [BL, T, F]
        xTc = np.ascontiguousarray(xc.transpose(2, 1, 0)).astype(BF16)
        maps.append({"xT": xTc, **shared})
    return maps


def run(inputs, trace=False):
    nc = _get_nc()
    maps = _prep_maps(**inputs)
    res = run_bass_kernel_spmd(nc, maps, list(range(N_CORES)), trace=trace)
    outs = [np.asarray(res.results[i]["out"], np.float32) for i in range(N_CORES)]
    full = np.concatenate(outs, axis=0)            # [64, 1]
    return full, res.exec_time_ns


def kernel(**inputs):
    inputs = {k: np.asarray(v, np.float32) for k, v in inputs.items()}
    out_, _ = run(inputs, trace=False)
    return out_


# revision 3
# speedup vs baseline: 1.0040x; 1.0040x over previous
"""Trainium2 Bass kernel for a 2-layer GRU time-series binary classifier, v2.

Model (torch GRU semantics, batch_first):
  seq1, _ = GRU(F=2048 -> H1=128)(x)        x: [64, 512, 2048]
  _,  h2 = GRU(H1 -> H2=64)(seq1)
  out = h2 @ fc_w.T + fc_b                  -> [64, 1]

Data-parallel over batch across 8 cores (8 sequences each).  The wall
time is the 512-step recurrence chain latency, so the per-step
cross-engine dependency chain is minimized:

  PE(3 bf16 matmuls) -> ACT sigmoid(r|zeta) -> DVE stt -> DVE tt
     -> ACT tanh -> DVE tensor_tensor_scan -> next PE

Key structure:
  * z' = 1-z comes free as sigmoid(-zeta) via the ACT scale port.
  * h' = n*(1-z) + z*h is ONE DVE op: a hardware prefix scan over
    per-sequence column pairs d0=[0, n], d1=[z', z*h]; the zero in d0
    kills the carried state at each pair boundary, so
    out[odd] = n*z' + z*h exactly.  z*h is computed off the critical
    path on DVE as soon as sigmoid lands (never GpSimd: its semaphore
    instructions cost ~1.3us each on HW).
  * All matmul operands are bf16 (h history stored bf16) so LDWEIGHTS
    costs halve; accumulation stays fp32 in PSUM, gates/cell fp32.
  * PSUM discipline: a start=True matmul clears has_written bits for
    its WHOLE bank, and the tile framework only tracks region-level
    deps, so every start=True producer (per-chunk GEMM rz, per-step
    hn, per-layer xn scratch) owns a full, exclusive bank.
Layer 2 runs LAG slots behind layer 1, brick-laid one engine-phase
behind inside each slot; the chunked input GEMMs ride as low-priority
thunks in the PE's idle windows.
"""

import numpy as np
import ml_dtypes

from concourse import bacc, tile, mybir
from concourse.bass_utils import run_bass_kernel_spmd

BF16 = ml_dtypes.bfloat16
N_CORES = 8
B, T, F = 64, 512, 2048
import os as _os
T = int(_os.environ.get("BASS_GRU_T", T))   # shrink for simulator checks
H1, H2 = 128, 64
BL = B // N_CORES             # 8 sequences per core
CHUNK = 32                    # timesteps per GEMM chunk
NCH = T // CHUNK              # 16 chunks
NW = CHUNK * BL               # 256 moving columns per chunk GEMM
KT = F // 128                 # 16 K-tiles for GEMM1
LAG = 40                      # layer-2 slot lag
R = 8                         # step ring size
AF = mybir.ActivationFunctionType
ALU = mybir.AluOpType
DT_BF = mybir.dt.bfloat16
DT_F32 = mybir.dt.float32


def build_nc():
    nc = bacc.Bacc(None, target_bir_lowering=False)

    xT = nc.declare_dram_parameter("xT", [F, T, BL], DT_BF, isOutput=False)
    wih1T = nc.declare_dram_parameter("wih1T", [F, 3 * H1], DT_BF, isOutput=False)
    whh1T = nc.declare_dram_parameter("whh1T", [H1, 3 * H1], DT_BF, isOutput=False)
    wih2T = nc.declare_dram_parameter("wih2T", [H1, 3 * H2], DT_BF, isOutput=False)
    whh2T = nc.declare_dram_parameter("whh2T", [H2, 3 * H2], DT_BF, isOutput=False)
    brow1 = nc.declare_dram_parameter("brow1", [1, 3 * H1], DT_F32, isOutput=False)
    brow2 = nc.declare_dram_parameter("brow2", [1, 3 * H2], DT_F32, isOutput=False)
    bhn1 = nc.declare_dram_parameter("bhn1", [H1, 1], DT_F32, isOutput=False)
    bhn2 = nc.declare_dram_parameter("bhn2", [H2, 1], DT_F32, isOutput=False)
    fcwT = nc.declare_dram_parameter("fcwT", [H2, 1], DT_F32, isOutput=False)
    fcb = nc.declare_dram_parameter("fcb", [BL, 1], DT_F32, isOutput=False)
    out = nc.declare_dram_parameter("out", [BL, 1], DT_F32, isOutput=True)

    with tile.TileContext(nc) as tc:
        with (
            tc.tile_pool(name="const", bufs=1) as cpool,
            tc.tile_pool(name="xchunk", bufs=3) as xpool,
            tc.tile_pool(name="g1p", bufs=2, space="PSUM") as g1pool,
            tc.tile_pool(name="g2p", bufs=2, space="PSUM") as g2pool,
            tc.tile_pool(name="scr", bufs=1, space="PSUM") as scrpool,
            tc.tile_pool(name="pfix", bufs=1, space="PSUM") as pfix,
        ):
            # ---- persistent tiles -------------------------------------
            w1 = cpool.tile([128, KT, 3 * H1], DT_BF)      # GEMM1 stationaries
            wh1 = cpool.tile([H1, 3 * H1], DT_BF)
            w2 = cpool.tile([H1, 3 * H2], DT_BF)
            wh2 = cpool.tile([H2, 3 * H2], DT_BF)
            br1 = cpool.tile([1, 3 * H1], DT_F32)
            br2 = cpool.tile([1, 3 * H2], DT_F32)
            bn1 = cpool.tile([H1, 1], DT_F32)
            bn2 = cpool.tile([H2, 1], DT_F32)
            fw = cpool.tile([H2, 1], DT_F32)
            fb = cpool.tile([BL, 1], DT_F32)
            onesf = cpool.tile([1, NW], DT_F32)
            xn1s = cpool.tile([H1, 2, NW], DT_F32)         # drained xn, A/B
            xn2s = cpool.tile([H2, 2, NW], DT_F32)

            # h1 history: per step a [2, BL] pair block; odd cols hold h.
            h1h = cpool.tile([H1, (T + 1) * 2 * BL], DT_BF)
            h2r = cpool.tile([H2, R * 2 * BL], DT_BF)      # h2 ring
            # step rings (slot = step % R)
            trz1 = cpool.tile([H1, R, 2, BL], DT_F32)      # [r, z]
            trz2 = cpool.tile([H2, R, 2, BL], DT_F32)
            d0r1 = cpool.tile([H1, R * 2 * BL], DT_F32)    # [0, n]
            d0r2 = cpool.tile([H2, R * 2 * BL], DT_F32)
            d1r1 = cpool.tile([H1, R * 2 * BL], DT_F32)    # [z', z*h]
            d1r2 = cpool.tile([H2, R * 2 * BL], DT_F32)
            m1 = cpool.tile([H1, R, BL], DT_F32)           # (hn+bn)*r
            m2 = cpool.tile([H2, R, BL], DT_F32)
            h2fin = cpool.tile([H2, BL], DT_F32)
            res = cpool.tile([BL, 1], DT_F32)

            # recurrent-n rings (slot = step % RN), one bank per layer:
            # a start=True matmul clears has_written for its WHOLE bank, so
            # the two layers' per-step fresh matmuls must not share one.
            RN = 32
            pband1 = pfix.tile([H1, 512], DT_F32)
            pband2 = pfix.tile([H2, 512], DT_F32)
            hn1 = pband1[:, 0:RN * BL]
            hn2 = pband2[:, 0:RN * BL]
            fcp = pband2[0:BL, RN * BL + 128:RN * BL + 129]
            # per-layer scratch banks for the xn GEMM outputs (a start=True
            # matmul wipes its whole bank, so the layers must not share one;
            # full-bank sized so the allocator cannot co-locate them)
            scr1 = scrpool.tile([H1, 512], DT_F32)
            scr2 = scrpool.tile([H2, 512], DT_F32)

            nc.sync.dma_start(out=w1[:], in_=wih1T.rearrange("(kt p) g -> p kt g", p=128))
            nc.sync.dma_start(out=wh1[:], in_=whh1T[:])
            nc.sync.dma_start(out=w2[:], in_=wih2T[:])
            nc.sync.dma_start(out=wh2[:], in_=whh2T[:])
            nc.sync.dma_start(out=br1[:], in_=brow1[:])
            nc.sync.dma_start(out=br2[:], in_=brow2[:])
            nc.sync.dma_start(out=bn1[:], in_=bhn1[:])
            nc.sync.dma_start(out=bn2[:], in_=bhn2[:])
            nc.sync.dma_start(out=fw[:], in_=fcwT[:])
            nc.sync.dma_start(out=fb[:], in_=fcb[:])
            nc.vector.memset(onesf[:], 1.0)
            nc.vector.memset(d0r1[:], 0.0)   # even cols must stay 0 forever
            nc.vector.memset(d0r2[:], 0.0)
            nc.vector.memset(h1h[:, 0:2 * BL], 0.0)   # h1(-1) = 0
            nc.vector.memset(h2r[:, 0:2 * BL], 0.0)   # h2(-1) = 0

            # paired views: [..., region/slot, b, {even,odd}]
            h1v = h1h.rearrange("p (g b two) -> p g b two", b=BL, two=2)
            h2v = h2r.rearrange("p (g b two) -> p g b two", b=BL, two=2)
            d0v1 = d0r1.rearrange("p (g b two) -> p g b two", b=BL, two=2)
            d0v2 = d0r2.rearrange("p (g b two) -> p g b two", b=BL, two=2)
            d1v1 = d1r1.rearrange("p (g b two) -> p g b two", b=BL, two=2)
            d1v2 = d1r2.rearrange("p (g b two) -> p g b two", b=BL, two=2)

            xtiles = {}
            rz1_ps = {}
            rz2_ps = {}
            xn1_ps = {}
            xn2_ps = {}

            def dma_xchunk(c):
                # split across two HWDGE rings (SP + ACT) so the ~47us/MB
                # single-DMA-engine transfer doesn't starve the GEMM riders
                xt = xpool.tile([128, KT, NW], DT_BF, tag="xc")
                src = xT[:, c * CHUNK:(c + 1) * CHUNK, :].rearrange(
                    "(kt p) t b -> p kt (t b)", p=128)
                nc.sync.dma_start(out=xt[:, 0:KT // 2], in_=src[:, 0:KT // 2])
                nc.scalar.dma_start(out=xt[:, KT // 2:KT],
                                    in_=src[:, KT // 2:KT])
                xtiles[c] = xt

            def gemm1_closures(c):
                """Layer-1 input projection of chunk c -> list of thunks."""
                rz = g1pool.tile([H1, 2 * NW], DT_F32, tag="g1")
                xn = scr1[:, 0:NW]
                rz1_ps[c], xn1_ps[c] = rz, xn1s[:, c % 2]
                xt = xtiles[c]
                thunks = []
                for g, dst, st0 in ((0, rz[:, 0:NW], True), (1, rz[:, NW:2 * NW], False),
                                    (2, xn[:, 0:NW], True)):
                    def mk(kt, g=g, dst=dst, st0=st0):
                        def f():
                            # only a bank's FIRST matmul may set start=True
                            nc.tensor.matmul(
                                dst, w1[:, kt, g * 128:(g + 1) * 128], xt[:, kt],
                                start=(st0 and kt == 0), stop=False,
                                skip_group_check=True)
                        return f
                    for kt in range(KT):
                        thunks.append(mk(kt))

                    def fbias(g=g, dst=dst):
                        nc.tensor.matmul(
                            dst, br1[:, g * 128:(g + 1) * 128], onesf[:],
                            start=False, stop=True, skip_group_check=True)
                    thunks.append(fbias)

                def fdrain():
                    nc.scalar.copy(xn1s[:, c % 2], xn[:, 0:NW])
                thunks.append(fdrain)
                return thunks

            def gemm2_closures(c):
                """Layer-2 input projection of chunk c (reads h1 history)."""
                rz = g2pool.tile([H2, 2 * NW], DT_F32, tag="g2")
                xn = scr2[:, 0:NW]
                rz2_ps[c], xn2_ps[c] = rz, xn2s[:, c % 2]
                mv = h1v[:, c * CHUNK + 1:(c + 1) * CHUNK + 1, :, 1]  # [H1,32,8]
                thunks = []
                for g, dst, st0 in ((0, rz[:, 0:NW], True), (1, rz[:, NW:2 * NW], False),
                                    (2, xn[:, 0:NW], True)):
                    def fmm(g=g, dst=dst, st0=st0):
                        nc.tensor.matmul(
                            dst, w2[:, g * H2:(g + 1) * H2], mv,
                            start=st0, stop=False, skip_group_check=True)
                    thunks.append(fmm)

                    def fbias(g=g, dst=dst):
                        nc.tensor.matmul(
                            dst, br2[:, g * H2:(g + 1) * H2], onesf[:],
                            start=False, stop=True, skip_group_check=True)
                    thunks.append(fbias)

                def fdrain():
                    nc.scalar.copy(xn2s[:, c % 2], xn[:, 0:NW])
                thunks.append(fdrain)
                return thunks

            def step_phases(layer, s):
                c, t, k32, k = s // CHUNK, s % CHUNK, s % RN, s % R
                if layer == 1:
                    H, wh, bn, rz, xn, hn = H1, wh1, bn1, rz1_ps[c], xn1_ps[c], hn1
                    trz, d0, d0v, d1, d1v, mt = trz1, d0r1, d0v1, d1r1, d1v1, m1
                    hprev = h1v[:, s, :, 1]                # [H, BL] h(s-1)
                    hout = h1h[:, (s + 1) * 2 * BL:(s + 2) * 2 * BL]
                else:
                    H, wh, bn, rz, xn, hn = H2, wh2, bn2, rz2_ps[c], xn2_ps[c], hn2
                    trz, d0, d0v, d1, d1v, mt = trz2, d0r2, d0v2, d1r2, d1v2, m2
                    hprev = h2v[:, s % R, :, 1]
                    hout = h2r[:, ((s + 1) % R) * 2 * BL:(((s + 1) % R) + 1) * 2 * BL]
                hnsl = hn[:, k32 * BL:(k32 + 1) * BL]
                rsl = slice(t * BL, (t + 1) * BL)
                zsl = slice(NW + t * BL, NW + (t + 1) * BL)

                def ph_mm():
                    nc.tensor.matmul(rz[:, rsl], wh[:, 0:H], hprev,
                                     start=False, stop=True, skip_group_check=True)
                    nc.tensor.matmul(rz[:, zsl], wh[:, H:2 * H], hprev,
                                     start=False, stop=True, skip_group_check=True)
                    nc.tensor.matmul(hnsl, wh[:, 2 * H:3 * H], hprev,
                                     start=True, stop=True, skip_group_check=True)

                def ph_sig():
                    rzv = rz.rearrange("p (g x) -> p g x", g=2)[:, :, rsl]
                    nc.scalar.activation(trz[:, k], rzv, AF.Sigmoid)
                    # z' = 1 - z = sigmoid(-zeta) into d1 even cols
                    nc.scalar.activation(d1v[:, k, :, 0], rz[:, zsl],
                                         AF.Sigmoid, scale=-1.0)

                def ph_b():
                    # z*h into d1 odd cols (off critical path)
                    nc.vector.tensor_tensor(out=d1v[:, k, :, 1], in0=trz[:, k, 1],
                                            in1=hprev, op=ALU.mult)

                def ph_mul():
                    nc.vector.scalar_tensor_tensor(
                        out=mt[:, k], in0=hnsl, scalar=bn[:], in1=trz[:, k, 0],
                        op0=ALU.add, op1=ALU.mult)
                    nc.vector.tensor_tensor(out=hnsl, in0=mt[:, k],
                                            in1=xn[:, rsl], op=ALU.add)

                def ph_tanh():
                    nc.scalar.activation(d0v[:, k, :, 1], hnsl, AF.Tanh)

                def ph_scan():
                    nc.vector.tensor_tensor_scan(
                        out=hout, data0=d0[:, k * 2 * BL:(k + 1) * 2 * BL],
                        data1=d1[:, k * 2 * BL:(k + 1) * 2 * BL],
                        initial=0.0, op0=ALU.mult, op1=ALU.add)

                return {"mm": ph_mm, "sig": ph_sig, "b": ph_b, "mul": ph_mul,
                        "tanh": ph_tanh, "scan": ph_scan}

            # ---- prologue --------------------------------------------
            dma_xchunk(0)
            dma_xchunk(1)
            for f in gemm1_closures(0):
                f()

            # ---- flat slot timeline ----------------------------------
            thunks = []
            for s in range(T + LAG):
                if s % CHUNK == 0:
                    c = s // CHUNK
                    if 1 <= c <= NCH:
                        thunks += gemm2_closures(c - 1)
                    if c + 2 < NCH:
                        thunks.append(lambda c=c: dma_xchunk(c + 2))
                    if c + 1 < NCH:
                        thunks += gemm1_closures(c + 1)
                u = s - LAG
                p1 = step_phases(1, s) if s < T else None
                p2 = step_phases(2, u) if 0 <= u < T else None
                # chain ops get priority 0 so the scheduler never parks a
                # GEMM rider between a step's scan and the next step's matmul
                with tc.high_priority():
                    for ph in ("mm", "sig", "mul", "b", "tanh", "scan"):
                        if p1:
                            p1[ph]()
                        if p2:
                            p2[ph]()
                for _ in range(2):
                    if thunks:
                        thunks.pop(0)()
            while thunks:
                thunks.pop(0)()

            # ---- fc head ---------------------------------------------
            nc.scalar.copy(h2fin[:], h2v[:, T % R, :, 1])
            nc.tensor.matmul(fcp, h2fin[:], fw[:], start=True, stop=True,
                             skip_group_check=True)
            nc.scalar.activation(res[:], fcp, AF.Identity, bias=fb[:])
            nc.sync.dma_start(out=out[:], in_=res[:])

    nc.compile()
    return nc


_NC_CACHE = {}


def _get_nc():
    if "nc" not in _NC_CACHE:
        _NC_CACHE["nc"] = build_nc()
    return _NC_CACHE["nc"]


def _prep_maps(x, w_ih1, w_hh1, b_ih1, b_hh1, w_ih2, w_hh2, b_ih2, b_hh2,
               fc_w, fc_b):
    f32 = np.float32
    brow1v = np.concatenate([
        (b_ih1[:H1] + b_hh1[:H1]),
        (b_ih1[H1:2 * H1] + b_hh1[H1:2 * H1]),
        b_ih1[2 * H1:],
    ]).reshape(1, 3 * H1)
    brow2v = np.concatenate([
        (b_ih2[:H2] + b_hh2[:H2]),
        (b_ih2[H2:2 * H2] + b_hh2[H2:2 * H2]),
        b_ih2[2 * H2:],
    ]).reshape(1, 3 * H2)
    shared = {
        "wih1T": np.ascontiguousarray(w_ih1.T).astype(BF16),
        "whh1T": np.ascontiguousarray(w_hh1.T).astype(BF16),
        "wih2T": np.ascontiguousarray(w_ih2.T).astype(BF16),
        "whh2T": np.ascontiguousarray(w_hh2.T).astype(BF16),
        "brow1": brow1v.astype(f32),
        "brow2": brow2v.astype(f32),
        "bhn1": np.ascontiguousarray(b_hh1[2 * H1:].reshape(H1, 1), dtype=f32),
        "bhn2": np.ascontiguousarray(b_hh2[2 * H2:].reshape(H2, 1), dtype=f32),
        "fcwT": np.ascontiguousarray(fc_w.reshape(1, H2).T, dtype=f32),
        "fcb": np.full((BL, 1), float(np.asarray(fc_b).reshape(-1)[0]), dtype=f32),
    }
    maps = []
    for c in range(N_CORES):
        xc = x[c * BL:(c + 1) * BL]          # [BL, T, F]
        xTc = np.ascontiguousarray(xc.transpose(2, 1, 0)).astype(BF16)
        maps.append({"xT": xTc, **shared})
    return maps


def run(inputs, trace=False):
    nc = _get_nc()
    maps = _prep_maps(**inputs)
    res = run_bass_kernel_spmd(nc, maps, list(range(N_CORES)), trace=trace)
    outs = [np.asarray(res.results[i]["out"], np.float32) for i in range(N_CORES)]
    full = np.concatenate(outs, axis=0)            # [64, 1]
    return full, res.exec_time_ns


def kernel(**inputs):
    inputs = {k: np.asarray(v, np.float32) for k, v in inputs.items()}
    out_, _ = run(inputs, trace=False)
    return out_


# revision 4
# speedup vs baseline: 1.0057x; 1.0017x over previous
"""Trainium2 Bass kernel for a 2-layer GRU time-series binary classifier, v2.

Model (torch GRU semantics, batch_first):
  seq1, _ = GRU(F=2048 -> H1=128)(x)        x: [64, 512, 2048]
  _,  h2 = GRU(H1 -> H2=64)(seq1)
  out = h2 @ fc_w.T + fc_b                  -> [64, 1]

Data-parallel over batch across 8 cores (8 sequences each).  The wall
time is the 512-step recurrence chain latency, so the per-step
cross-engine dependency chain is minimized:

  PE(3 bf16 matmuls) -> ACT sigmoid(r|zeta) -> DVE stt -> DVE tt
     -> ACT tanh -> DVE tensor_tensor_scan -> next PE

Key structure:
  * z' = 1-z comes free as sigmoid(-zeta) via the ACT scale port.
  * h' = n*(1-z) + z*h is ONE DVE op: a hardware prefix scan over
    per-sequence column pairs d0=[0, n], d1=[z', z*h]; the zero in d0
    kills the carried state at each pair boundary, so
    out[odd] = n*z' + z*h exactly.  z*h is computed off the critical
    path on DVE as soon as sigmoid lands (never GpSimd: its semaphore
    instructions cost ~1.3us each on HW).
  * All matmul operands are bf16 (h history stored bf16) so LDWEIGHTS
    costs halve; accumulation stays fp32 in PSUM, gates/cell fp32.
  * PSUM discipline: a start=True matmul clears has_written bits for
    its WHOLE bank, and the tile framework only tracks region-level
    deps, so every start=True producer (per-chunk GEMM rz, per-step
    hn, per-layer xn scratch) owns a full, exclusive bank.
Layer 2 runs LAG slots behind layer 1, brick-laid one engine-phase
behind inside each slot; the chunked input GEMMs ride as low-priority
thunks in the PE's idle windows.
"""

import numpy as np
import ml_dtypes

from concourse import bacc, tile, mybir
from concourse.bass_utils import run_bass_kernel_spmd

BF16 = ml_dtypes.bfloat16
N_CORES = 8
B, T, F = 64, 512, 2048
import os as _os
T = int(_os.environ.get("BASS_GRU_T", T))   # shrink for simulator checks
H1, H2 = 128, 64
BL = B // N_CORES             # 8 sequences per core
CHUNK = 32                    # timesteps per GEMM chunk
NCH = T // CHUNK              # 16 chunks
NW = CHUNK * BL               # 256 moving columns per chunk GEMM
KT = F // 128                 # 16 K-tiles for GEMM1
LAG = 40                      # layer-2 slot lag
R = 8                         # step ring size
AF = mybir.ActivationFunctionType
ALU = mybir.AluOpType
DT_BF = mybir.dt.bfloat16
DT_F32 = mybir.dt.float32


def build_nc():
    nc = bacc.Bacc(None, target_bir_lowering=False)

    xT = nc.declare_dram_parameter("xT", [F, T, BL], DT_BF, isOutput=False)
    wih1T = nc.declare_dram_parameter("wih1T", [F, 3 * H1], DT_BF, isOutput=False)
    whh1T = nc.declare_dram_parameter("whh1T", [H1, 3 * H1], DT_BF, isOutput=False)
    wih2T = nc.declare_dram_parameter("wih2T", [H1, 3 * H2], DT_BF, isOutput=False)
    whh2T = nc.declare_dram_parameter("whh2T", [H2, 3 * H2], DT_BF, isOutput=False)
    brow1 = nc.declare_dram_parameter("brow1", [1, 3 * H1], DT_F32, isOutput=False)
    brow2 = nc.declare_dram_parameter("brow2", [1, 3 * H2], DT_F32, isOutput=False)
    bhn1 = nc.declare_dram_parameter("bhn1", [H1, 1], DT_F32, isOutput=False)
    bhn2 = nc.declare_dram_parameter("bhn2", [H2, 1], DT_F32, isOutput=False)
    fcwT = nc.declare_dram_parameter("fcwT", [H2, 1], DT_F32, isOutput=False)
    fcb = nc.declare_dram_parameter("fcb", [BL, 1], DT_F32, isOutput=False)
    out = nc.declare_dram_parameter("out", [BL, 1], DT_F32, isOutput=True)

    with tile.TileContext(nc) as tc:
        with (
            tc.tile_pool(name="const", bufs=1) as cpool,
            tc.tile_pool(name="xchunk", bufs=3) as xpool,
            tc.tile_pool(name="g1p", bufs=2, space="PSUM") as g1pool,
            tc.tile_pool(name="g2p", bufs=2, space="PSUM") as g2pool,
            tc.tile_pool(name="scr", bufs=1, space="PSUM") as scrpool,
            tc.tile_pool(name="pfix", bufs=1, space="PSUM") as pfix,
        ):
            # ---- persistent tiles -------------------------------------
            w1 = cpool.tile([128, KT, 3 * H1], DT_BF)      # GEMM1 stationaries
            wh1 = cpool.tile([H1, 3 * H1], DT_BF)
            w2 = cpool.tile([H1, 3 * H2], DT_BF)
            wh2 = cpool.tile([H2, 3 * H2], DT_BF)
            br1 = cpool.tile([1, 3 * H1], DT_F32)
            br2 = cpool.tile([1, 3 * H2], DT_F32)
            bn1 = cpool.tile([H1, 1], DT_F32)
            bn2 = cpool.tile([H2, 1], DT_F32)
            fw = cpool.tile([H2, 1], DT_F32)
            fb = cpool.tile([BL, 1], DT_F32)
            onesf = cpool.tile([1, NW], DT_F32)
            xn1s = cpool.tile([H1, 2, NW], DT_F32)         # drained xn, A/B
            xn2s = cpool.tile([H2, 2, NW], DT_F32)

            # h1 history: per step a [2, BL] pair block; odd cols hold h.
            h1h = cpool.tile([H1, (T + 1) * 2 * BL], DT_BF)
            h2r = cpool.tile([H2, R * 2 * BL], DT_BF)      # h2 ring
            # step rings (slot = step % R)
            trz1 = cpool.tile([H1, R, 2, BL], DT_F32)      # [r, z]
            trz2 = cpool.tile([H2, R, 2, BL], DT_F32)
            d0r1 = cpool.tile([H1, R * 2 * BL], DT_F32)    # [0, n]
            d0r2 = cpool.tile([H2, R * 2 * BL], DT_F32)
            d1r1 = cpool.tile([H1, R * 2 * BL], DT_F32)    # [z', z*h]
            d1r2 = cpool.tile([H2, R * 2 * BL], DT_F32)
            m1 = cpool.tile([H1, R, BL], DT_F32)           # (hn+bn)*r
            m2 = cpool.tile([H2, R, BL], DT_F32)
            h2fin = cpool.tile([H2, BL], DT_F32)
            res = cpool.tile([BL, 1], DT_F32)

            # recurrent-n rings (slot = step % RN), one bank per layer:
            # a start=True matmul clears has_written for its WHOLE bank, so
            # the two layers' per-step fresh matmuls must not share one.
            RN = 32
            pband1 = pfix.tile([H1, 512], DT_F32)
            pband2 = pfix.tile([H2, 512], DT_F32)
            hn1 = pband1[:, 0:RN * BL]
            hn2 = pband2[:, 0:RN * BL]
            fcp = pband2[0:BL, RN * BL + 128:RN * BL + 129]
            # per-layer scratch banks for the xn GEMM outputs (a start=True
            # matmul wipes its whole bank, so the layers must not share one;
            # full-bank sized so the allocator cannot co-locate them)
            scr1 = scrpool.tile([H1, 512], DT_F32)
            scr2 = scrpool.tile([H2, 512], DT_F32)

            nc.sync.dma_start(out=w1[:], in_=wih1T.rearrange("(kt p) g -> p kt g", p=128))
            nc.sync.dma_start(out=wh1[:], in_=whh1T[:])
            nc.sync.dma_start(out=w2[:], in_=wih2T[:])
            nc.sync.dma_start(out=wh2[:], in_=whh2T[:])
            nc.sync.dma_start(out=br1[:], in_=brow1[:])
            nc.sync.dma_start(out=br2[:], in_=brow2[:])
            nc.sync.dma_start(out=bn1[:], in_=bhn1[:])
            nc.sync.dma_start(out=bn2[:], in_=bhn2[:])
            nc.sync.dma_start(out=fw[:], in_=fcwT[:])
            nc.sync.dma_start(out=fb[:], in_=fcb[:])
            nc.vector.memset(onesf[:], 1.0)
            nc.vector.memset(d0r1[:], 0.0)   # even cols must stay 0 forever
            nc.vector.memset(d0r2[:], 0.0)
            nc.vector.memset(h1h[:, 0:2 * BL], 0.0)   # h1(-1) = 0
            nc.vector.memset(h2r[:, 0:2 * BL], 0.0)   # h2(-1) = 0

            # paired views: [..., region/slot, b, {even,odd}]
            h1v = h1h.rearrange("p (g b two) -> p g b two", b=BL, two=2)
            h2v = h2r.rearrange("p (g b two) -> p g b two", b=BL, two=2)
            d0v1 = d0r1.rearrange("p (g b two) -> p g b two", b=BL, two=2)
            d0v2 = d0r2.rearrange("p (g b two) -> p g b two", b=BL, two=2)
            d1v1 = d1r1.rearrange("p (g b two) -> p g b two", b=BL, two=2)
            d1v2 = d1r2.rearrange("p (g b two) -> p g b two", b=BL, two=2)

            xtiles = {}
            rz1_ps = {}
            rz2_ps = {}
            xn1_ps = {}
            xn2_ps = {}

            def dma_xchunk(c):
                # split across two HWDGE rings (SP + ACT) so the ~47us/MB
                # single-DMA-engine transfer doesn't starve the GEMM riders
                xt = xpool.tile([128, KT, NW], DT_BF, tag="xc")
                src = xT[:, c * CHUNK:(c + 1) * CHUNK, :].rearrange(
                    "(kt p) t b -> p kt (t b)", p=128)
                nc.sync.dma_start(out=xt[:, 0:KT // 2], in_=src[:, 0:KT // 2])
                nc.scalar.dma_start(out=xt[:, KT // 2:KT],
                                    in_=src[:, KT // 2:KT])
                xtiles[c] = xt

            def gemm1_closures(c):
                """Layer-1 input projection of chunk c -> list of thunks."""
                rz = g1pool.tile([H1, 2 * NW], DT_F32, tag="g1")
                xn = scr1[:, 0:NW]
                rz1_ps[c], xn1_ps[c] = rz, xn1s[:, c % 2]
                xt = xtiles[c]
                thunks = []
                for g, dst, st0 in ((0, rz[:, 0:NW], True), (1, rz[:, NW:2 * NW], False),
                                    (2, xn[:, 0:NW], True)):
                    def mk(kt, half, g=g, dst=dst, st0=st0):
                        # halved moving width caps the damage when a cold
                        # rider parks ahead of the recurrence chain's matmul
                        lo, hi = half * (NW // 2), (half + 1) * (NW // 2)
                        def f():
                            # only a bank's FIRST matmul may set start=True
                            nc.tensor.matmul(
                                dst[:, lo:hi],
                                w1[:, kt, g * 128:(g + 1) * 128],
                                xt[:, kt][:, lo:hi],
                                start=(st0 and kt == 0 and half == 0),
                                stop=False, skip_group_check=True)
                        return f
                    for kt in range(KT):
                        thunks.append(mk(kt, 0))
                        thunks.append(mk(kt, 1))

                    def fbias(g=g, dst=dst):
                        nc.tensor.matmul(
                            dst, br1[:, g * 128:(g + 1) * 128], onesf[:],
                            start=False, stop=True, skip_group_check=True)
                    thunks.append(fbias)

                def fdrain():
                    nc.vector.tensor_copy(out=xn1s[:, c % 2], in_=xn[:, 0:NW])
                thunks.append(fdrain)
                return thunks

            def gemm2_closures(c):
                """Layer-2 input projection of chunk c (reads h1 history)."""
                rz = g2pool.tile([H2, 2 * NW], DT_F32, tag="g2")
                xn = scr2[:, 0:NW]
                rz2_ps[c], xn2_ps[c] = rz, xn2s[:, c % 2]
                mv = h1v[:, c * CHUNK + 1:(c + 1) * CHUNK + 1, :, 1]  # [H1,32,8]
                thunks = []
                for g, dst, st0 in ((0, rz[:, 0:NW], True), (1, rz[:, NW:2 * NW], False),
                                    (2, xn[:, 0:NW], True)):
                    def fmm(half, g=g, dst=dst, st0=st0):
                        lo, hi = half * (NW // 2), (half + 1) * (NW // 2)
                        def f():
                            nc.tensor.matmul(
                                dst[:, lo:hi], w2[:, g * H2:(g + 1) * H2],
                                mv[:, half * (CHUNK // 2):(half + 1) * (CHUNK // 2)],
                                start=(st0 and half == 0), stop=False,
                                skip_group_check=True)
                        return f
                    thunks.append(fmm(0))
                    thunks.append(fmm(1))

                    def fbias(g=g, dst=dst):
                        nc.tensor.matmul(
                            dst, br2[:, g * H2:(g + 1) * H2], onesf[:],
                            start=False, stop=True, skip_group_check=True)
                    thunks.append(fbias)

                def fdrain():
                    nc.vector.tensor_copy(out=xn2s[:, c % 2], in_=xn[:, 0:NW])
                thunks.append(fdrain)
                return thunks

            def step_phases(layer, s):
                c, t, k32, k = s // CHUNK, s % CHUNK, s % RN, s % R
                if layer == 1:
                    H, wh, bn, rz, xn, hn = H1, wh1, bn1, rz1_ps[c], xn1_ps[c], hn1
                    trz, d0, d0v, d1, d1v, mt = trz1, d0r1, d0v1, d1r1, d1v1, m1
                    hprev = h1v[:, s, :, 1]                # [H, BL] h(s-1)
                    hout = h1h[:, (s + 1) * 2 * BL:(s + 2) * 2 * BL]
                else:
                    H, wh, bn, rz, xn, hn = H2, wh2, bn2, rz2_ps[c], xn2_ps[c], hn2
                    trz, d0, d0v, d1, d1v, mt = trz2, d0r2, d0v2, d1r2, d1v2, m2
                    hprev = h2v[:, s % R, :, 1]
                    hout = h2r[:, ((s + 1) % R) * 2 * BL:(((s + 1) % R) + 1) * 2 * BL]
                hnsl = hn[:, k32 * BL:(k32 + 1) * BL]
                rsl = slice(t * BL, (t + 1) * BL)
                zsl = slice(NW + t * BL, NW + (t + 1) * BL)

                def ph_mm():
                    nc.tensor.matmul(rz[:, rsl], wh[:, 0:H], hprev,
                                     start=False, stop=True, skip_group_check=True)
                    nc.tensor.matmul(rz[:, zsl], wh[:, H:2 * H], hprev,
                                     start=False, stop=True, skip_group_check=True)
                    nc.tensor.matmul(hnsl, wh[:, 2 * H:3 * H], hprev,
                                     start=True, stop=True, skip_group_check=True)

                def ph_sig():
                    rzv = rz.rearrange("p (g x) -> p g x", g=2)[:, :, rsl]
                    nc.scalar.activation(trz[:, k], rzv, AF.Sigmoid)
                    # z' = 1 - z = sigmoid(-zeta) into d1 even cols
                    nc.scalar.activation(d1v[:, k, :, 0], rz[:, zsl],
                                         AF.Sigmoid, scale=-1.0)

                def ph_b():
                    # z*h into d1 odd cols (off critical path)
                    nc.vector.tensor_tensor(out=d1v[:, k, :, 1], in0=trz[:, k, 1],
                                            in1=hprev, op=ALU.mult)

                def ph_mul():
                    nc.vector.scalar_tensor_tensor(
                        out=mt[:, k], in0=hnsl, scalar=bn[:], in1=trz[:, k, 0],
                        op0=ALU.add, op1=ALU.mult)
                    nc.vector.tensor_tensor(out=hnsl, in0=mt[:, k],
                                            in1=xn[:, rsl], op=ALU.add)

                def ph_tanh():
                    nc.scalar.activation(d0v[:, k, :, 1], hnsl, AF.Tanh)

                def ph_scan():
                    nc.vector.tensor_tensor_scan(
                        out=hout, data0=d0[:, k * 2 * BL:(k + 1) * 2 * BL],
                        data1=d1[:, k * 2 * BL:(k + 1) * 2 * BL],
                        initial=0.0, op0=ALU.mult, op1=ALU.add)

                return {"mm": ph_mm, "sig": ph_sig, "b": ph_b, "mul": ph_mul,
                        "tanh": ph_tanh, "scan": ph_scan}

            # ---- prologue --------------------------------------------
            dma_xchunk(0)
            dma_xchunk(1)
            for f in gemm1_closures(0):
                f()

            # ---- flat slot timeline ----------------------------------
            thunks = []
            for s in range(T + LAG):
                if s % CHUNK == 0:
                    c = s // CHUNK
                    if 1 <= c <= NCH:
                        thunks += gemm2_closures(c - 1)
                    if c + 2 < NCH:
                        thunks.append(lambda c=c: dma_xchunk(c + 2))
                    if c + 1 < NCH:
                        thunks += gemm1_closures(c + 1)
                u = s - LAG
                p1 = step_phases(1, s) if s < T else None
                p2 = step_phases(2, u) if 0 <= u < T else None
                # chain ops get priority 0 so the scheduler never parks a
                # GEMM rider between a step's scan and the next step's matmul
                with tc.high_priority():
                    for ph in ("mm", "sig", "mul", "b", "tanh", "scan"):
                        if p1:
                            p1[ph]()
                        if p2:
                            p2[ph]()
                for _ in range(4):
                    if thunks:
                        thunks.pop(0)()
            while thunks:
                thunks.pop(0)()

            # ---- fc head ---------------------------------------------
            nc.scalar.copy(h2fin[:], h2v[:, T % R, :, 1])
            nc.tensor.matmul(fcp, h2fin[:], fw[:], start=True, stop=True,
                             skip_group_check=True)
            nc.scalar.activation(res[:], fcp, AF.Identity, bias=fb[:])
            nc.sync.dma_start(out=out[:], in_=res[:])

    nc.compile()
    return nc


_NC_CACHE = {}


def _get_nc():
    if "nc" not in _NC_CACHE:
        _NC_CACHE["nc"] = build_nc()
    return _NC_CACHE["nc"]


def _prep_maps(x, w_ih1, w_hh1, b_ih1, b_hh1, w_ih2, w_hh2, b_ih2, b_hh2,
               fc_w, fc_b):
    f32 = np.float32
    brow1v = np.concatenate([
        (b_ih1[:H1] + b_hh1[:H1]),
        (b_ih1[H1:2 * H1] + b_hh1[H1:2 * H1]),
        b_ih1[2 * H1:],
    ]).reshape(1, 3 * H1)
    brow2v = np.concatenate([
        (b_ih2[:H2] + b_hh2[:H2]),
        (b_ih2[H2:2 * H2] + b_hh2[H2:2 * H2]),
        b_ih2[2 * H2:],
    ]).reshape(1, 3 * H2)
    shared = {
        "wih1T": np.ascontiguousarray(w_ih1.T).astype(BF16),
        "whh1T": np.ascontiguousarray(w_hh1.T).astype(BF16),
        "wih2T": np.ascontiguousarray(w_ih2.T).astype(BF16),
        "whh2T": np.ascontiguousarray(w_hh2.T).astype(BF16),
        "brow1": brow1v.astype(f32),
        "brow2": brow2v.astype(f32),
        "bhn1": np.ascontiguousarray(b_hh1[2 * H1:].reshape(H1, 1), dtype=f32),
        "bhn2": np.ascontiguousarray(b_hh2[2 * H2:].reshape(H2, 1), dtype=f32),
        "fcwT": np.ascontiguousarray(fc_w.reshape(1, H2).T, dtype=f32),
        "fcb": np.full((BL, 1), float(np.asarray(fc_b).reshape(-1)[0]), dtype=f32),
    }
    maps = []
    for c in range(N_CORES):
        xc = x[c * BL:(c + 1) * BL]          # [BL, T, F]
        xTc = np.ascontiguousarray(xc.transpose(2, 1, 0)).astype(BF16)
        maps.append({"xT": xTc, **shared})
    return maps


def run(inputs, trace=False):
    nc = _get_nc()
    maps = _prep_maps(**inputs)
    res = run_bass_kernel_spmd(nc, maps, list(range(N_CORES)), trace=trace)
    outs = [np.asarray(res.results[i]["out"], np.float32) for i in range(N_CORES)]
    full = np.concatenate(outs, axis=0)            # [64, 1]
    return full, res.exec_time_ns


def kernel(**inputs):
    inputs = {k: np.asarray(v, np.float32) for k, v in inputs.items()}
    out_, _ = run(inputs, trace=False)
    return out_
